# revision 20
# baseline (speedup 1.0000x reference)
"""Causal self-attention (B=2, S=2048, E=1024, H=16) on 8 trn2 cores.

Sharding: batch x head -- core c handles batch c//4 and the 4 heads
starting at (c%4)*4. Each core runs QKV projection for its heads,
causal attention, and its slice of the output projection (row-split
c_proj); the host sums the 4 partial projections per batch.

Layout trick: scores are computed transposed (S^T[k, q]) so every
matmul streams N=512 moving columns, and the attention output comes
out as y^T[d, q] -- exactly the stationary operand the output
projection needs. Row-sums ride along as a ones-column appended to V.
All matmul inputs are float32r (~1e-4 rel err, full PE rate at N>=256).
"""

import os
import sys

import numpy as np

_DIR = os.path.dirname(os.path.abspath(__file__))
for _p in (_DIR,):
    if _p not in sys.path:
        sys.path.insert(0, _p)

import concourse.bass as bass
import concourse.mybir as mybir
from concourse import tile
from concourse.vector_clock import ScopedClock, VectorClock

F32 = mybir.dt.float32
F32R = mybir.dt.float32r

B, S, E, H, D = 2, 2048, 1024, 16, 64
HPC = 4          # heads per core
N_CORES = 8
QT = 512         # q tile (moving dim)
KC = 128         # k chunk (contraction tile)


class SplitDrainTileContext(tile.TileContext):
    """Kernel-tail drain with its sem waits split one per instruction.

    The walrus build here rejects instructions carrying more sync waits
    than their ISA struct encodes; TileContext hangs one wait per live
    proc on a single Drain. Sequential single-wait drains on the sync
    engine give the same guarantee.
    """

    def _drain_and_barrier(self, tick_clock, wait_clock):
        gc = list(tick_clock.global_clock)
        n = len(gc)
        for i, t in enumerate(gc):
            if t:
                vc = VectorClock([t if j == i else 0 for j in range(n)])
                inst = self.nc.sync.drain()
                wait_clock.add_sem_waits(inst.ins, ScopedClock({None: vc}))
        self.nc.all_engine_barrier()
        assert self.sems is not None
        popped = self.nc._tile_sem_poison_stack.pop()
        assert popped is self._sem_poison
        self.nc.clear_and_free_semaphores(list(self.sems.allocated().values()))
        self.nc.all_engine_barrier()


# ---------------------------------------------------------------- BIR fix

_CAPS = {"EventSemaphore": 2}
_DEFAULT_CAP = 1
_counter = [0]


def _split_bir_waits(bir):
    """Move excess sync waits onto EventSemaphores inserted just before
    the overloaded instruction (same engine => same program order)."""
    n = 0
    for fn in bir.get("functions", []):
        for bb in fn.get("blocks", []):
            out = []
            for inst in bb.get("instructions", []):
                si = inst.get("sync_info")
                waits = si.get("on_wait") if si else None
                cap = _CAPS.get(inst.get("opcode"), _DEFAULT_CAP)
                if waits and len(waits) > cap:
                    excess, keep = waits[:-cap], waits[-cap:]
                    for i in range(0, len(excess), 2):
                        _counter[0] += 1
                        out.append({
                            "debug": inst.get("debug", 0),
                            "engine": inst["engine"],
                            "ins": [], "outs": [],
                            "name": f"antsplitw-{_counter[0]}",
                            "opcode": "EventSemaphore",
                            "sync_info": {"on_update": [],
                                          "on_wait": excess[i:i + 2]},
                        })
                        n += 1
                    si["on_wait"] = keep
                out.append(inst)
            bb["instructions"] = out
    return n


def _install_bir_fix():
    import json
    import concourse.bass2jax as bass2jax
    from concourse.bass_utils import compile_bir_kernel as orig
    if getattr(bass2jax.compile_bir_kernel, "_ant_split", False):
        return

    def wrapped(ant_bir_str, *args, **kwargs):
        bir = json.loads(ant_bir_str)
        if _split_bir_waits(bir):
            ant_bir_str = json.dumps(bir).encode()
        return orig(ant_bir_str, *args, **kwargs)

    wrapped._ant_split = True
    bass2jax.compile_bir_kernel = wrapped


# ---------------------------------------------------------------- device

def build():
    nc = bass.Bass("TRN2", target_bir_lowering=False, debug=False)
    xT_d = nc.dram_tensor("xT", [E, S], F32R, kind="ExternalInput").ap()
    wqk_d = nc.dram_tensor("wqk", [E, 2 * HPC * D], F32R, kind="ExternalInput").ap()
    wv_d = nc.dram_tensor("wv", [E, HPC * D], F32R, kind="ExternalInput").ap()
    wp_d = nc.dram_tensor("wproj", [HPC * D, E], F32R, kind="ExternalInput").ap()
    y_d = nc.dram_tensor("y", [S, E], F32, kind="ExternalOutput").ap()

    EC = E // 128            # 8 contraction chunks over the embedding dim
    NQ = S // QT             # 4 q tiles
    NST = S // 128           # 16 s tiles of 128

    with SplitDrainTileContext(nc) as tc:
        with tc.tile_pool(name="persist", bufs=1) as persist:
            qT_sb = persist.tile([128, 2, S], F32R)    # heads 01 | 23 stacked
            kTpad = persist.tile([128, HPC, S], F32R)  # per head, rows 64+ zero
            vaug = persist.tile([128, NST, HPC, D + 1], F32R)
            yT = persist.tile([128, 2, S], F32R)       # normalized, proj lhsT
            wp_sb = persist.tile([128, 2, E], F32R)
            ones = persist.tile([128, 64], F32R)
            nc.vector.memset(ones[:].bitcast(F32), 1.0)
            nc.vector.memset(vaug[:, :, :, D:D + 1].bitcast(F32), 1.0)
            for h in range(HPC):
                dead = slice(64, 128) if h % 2 == 0 else slice(0, 64)
                nc.vector.memset(kTpad[dead, h, :].bitcast(F32), 0.0)

            # ---- phase 1: qkv projection (scoped input pool) ----
            with (
                tc.tile_pool(name="qkvin", bufs=1) as qkvin,
                tc.tile_pool(name="psq", bufs=4, space="PSUM") as psq,
                tc.tile_pool(name="psv", bufs=2, space="PSUM") as psv,
            ):
                xT_sb = qkvin.tile([128, EC, S], F32R)
                wqk_sb = qkvin.tile([128, EC, 512], F32R)
                wv_sb = qkvin.tile([128, EC, 256], F32R)
                # xT pieces stream column-block-major so the matmul emission
                # order below never head-of-line-blocks on a late chunk;
                # queues split so first-needed data has a dedicated engine
                def xt_piece(eng, ec, q4):
                    eng.dma_start(xT_sb[:, ec, q4 * 512:(q4 + 1) * 512],
                                  xT_d[ec * 128:(ec + 1) * 128, q4 * 512:(q4 + 1) * 512])
                for ec in range(EC):
                    nc.sync.dma_start(wqk_sb[:, ec, :], wqk_d[ec * 128:(ec + 1) * 128, :])
                    xt_piece(nc.scalar, ec, 0)
                for ec in range(EC):
                    nc.gpsimd.dma_start(wv_sb[:, ec, :], wv_d[ec * 128:(ec + 1) * 128, :])
                    xt_piece(nc.sync, ec, 1)
                for ec in range(EC):
                    xt_piece(nc.sync, ec, 2)
                    xt_piece(nc.gpsimd, ec, 3)
                for ci in range(2):
                    nc.scalar.dma_start(wp_sb[:, ci, :], wp_d[ci * 128:(ci + 1) * 128, :])

                def v_groups(q4):
                    # v natural: stationary = xT s-block, moving = wv
                    for st2 in range(4 * q4, 4 * q4 + 4):
                        ps = psv.tile([128, 256], F32)
                        for ec in range(EC):
                            nc.tensor.matmul(
                                ps[:],
                                xT_sb[:, ec, st2 * 128:(st2 + 1) * 128],
                                wv_sb[:, ec, :],
                                start=(ec == 0), stop=(ec == EC - 1))
                        nc.vector.tensor_copy(
                            out=vaug[:, st2, :, 0:D],
                            in_=ps[:, :].rearrange("p (h d) -> p h d", h=HPC))

                for q4 in range(4):
                    # q/k transposed: stationary = w column block, moving = xT
                    st = q4
                    sslc = slice(st * QT, (st + 1) * QT)
                    for rt in range(4):
                        ps = psq.tile([128, QT], F32)
                        for ec in range(EC):
                            nc.tensor.matmul(
                                ps[:],
                                wqk_sb[:, ec, rt * 128:(rt + 1) * 128],
                                xT_sb[:, ec, st * QT:(st + 1) * QT],
                                start=(ec == 0), stop=(ec == EC - 1))
                        if rt < 2:
                            nc.scalar.copy(qT_sb[:, rt, sslc], ps[:])
                        else:
                            # split the head pair into zero-padded per-head k,
                            # each head keeping its q's partition rows
                            nc.scalar.copy(kTpad[0:64, 2 * (rt - 2), sslc],
                                           ps[0:64, :])
                            nc.vector.tensor_copy(
                                out=kTpad[64:128, 2 * (rt - 2) + 1, sslc],
                                in_=ps[64:128, :])
                    if q4 > 0:
                        v_groups(q4 - 1)
                v_groups(3)

            # ---- phase 2: causal attention, transposed, unnormalized ----
            with tc.tile_pool(name="attw", bufs=1) as attw:
                # unnormalized y^T plus rowsums (row 64), one [65,512] slab
                # per (qj, h)
                yTun = attw.tile([65, NQ * HPC, QT], F32)
                attn_scope = (
                    tc.tile_pool(name="ptp", bufs=6),
                    tc.tile_pool(name="nrm", bufs=4),
                    tc.tile_pool(name="rts", bufs=8),
                    tc.tile_pool(name="bcs2", bufs=2),
                    tc.tile_pool(name="pout", bufs=3),
                    tc.tile_pool(name="pss", bufs=2, space="PSUM"),
                    tc.tile_pool(name="psav", bufs=2, space="PSUM"),
                    tc.tile_pool(name="psb", bufs=2, space="PSUM"),
                )
                (ptp, nrm, rts, bcs2, pout, pss, psav, psb) = (
                    p.__enter__() for p in attn_scope)
                ptp = ptp  # generator force
                attn_scope_entered = True

                def recip_half(qj, half):
                    # 1/rowsum for one head pair as exp(-ln(x)) on ACT:
                    # Ln reads the rowsum rows straight out of yTun (row 64)
                    # and the result lands at base partition 0, ready to be
                    # the bc matmul's moving operand -- no DMA hops at all.
                    t0 = qj * HPC + 2 * half
                    lg = nrm.tile([1, 2, QT], F32, tag="lg")
                    nc.scalar.activation(lg[:, :, :], yTun[64:65, t0:t0 + 2, :],
                                         mybir.ActivationFunctionType.Ln)
                    rt2 = rts.tile([1, 2, QT], F32R)
                    nc.scalar.activation(rt2[:, :, :], lg[:, :, :],
                                         mybir.ActivationFunctionType.Exp,
                                         scale=-1.0)
                    return [rt2[:, 0, :], rt2[:, 1, :]]

                def normmul(qj, rt_ts):
                    # normalize y^T for this q tile
                    qslc = slice(qj * QT, (qj + 1) * QT)
                    for h in range(HPC):
                        t = qj * HPC + h
                        rt_t = rt_ts[h]
                        bc = psb.tile([64, QT], F32, tag="pb")
                        nc.tensor.matmul(bc[:], ones[0:1, 0:64], rt_t[:, :],
                                         start=True, stop=True)
                        bc_sb = bcs2.tile([64, QT], F32)
                        nc.vector.tensor_copy(out=bc_sb[:], in_=bc[:])
                        po = 64 * (h % 2)
                        with nc.allow_low_precision(reason="proj lhsT"):
                            nc.vector.tensor_mul(yT[po:po + 64, h // 2, qslc],
                                                 yTun[0:64, t, :], bc_sb[:])

                def proj_slice(qj):
                    for qt in range(qj * 4, (qj + 1) * 4):
                        for eo in range(E // 512):
                            pp = psb.tile([128, 512], F32, tag="pb")
                            for ci in range(2):
                                nc.tensor.matmul(
                                    pp[:],
                                    yT[:, ci, qt * 128:(qt + 1) * 128],
                                    wp_sb[:, ci, eo * 512:(eo + 1) * 512],
                                    start=(ci == 0), stop=(ci == 1))
                            po_t = pout.tile([128, 512], F32)
                            nc.vector.tensor_copy(out=po_t[:], in_=pp[:])
                            nc.sync.dma_start(
                                y_d[qt * 128:(qt + 1) * 128,
                                    eo * 512:(eo + 1) * 512],
                                po_t[:])

                prev_qj = None
                for qj in range(NQ):
                    for h in range(HPC):
                        qslc = slice(qj * QT, (qj + 1) * QT)
                        qT_ap = qT_sb[:, h // 2, qslc]
                        nkc = (qj + 1) * QT // KC
                        av = psav.tile([65, QT], F32)
                        for pr in range(nkc // 2):
                            ps = pss.tile([128, 2, QT], F32)
                            offs = [max(0, (2 * pr + j) * KC - qj * QT)
                                    for j in range(2)]
                            for j in range(2):
                                kc = 2 * pr + j
                                o = offs[j]
                                nc.tensor.matmul(
                                    ps[:, j, o:QT],
                                    kTpad[:, h, kc * KC:(kc + 1) * KC],
                                    qT_ap[:, o:QT],
                                    start=True, stop=True)
                            pt = ptp.tile([128, 2, QT], F32R)
                            if offs[0] == 0 and offs[1] == 0:
                                nc.scalar.activation(
                                    pt[:], ps[:],
                                    mybir.ActivationFunctionType.Exp,
                                    scale=0.125)
                            else:
                                # diagonal pair: exp only live columns
                                for j, o in enumerate(offs):
                                    nc.scalar.activation(
                                        pt[:, j, o:QT], ps[:, j, o:QT],
                                        mybir.ActivationFunctionType.Exp,
                                        scale=0.125)
                            for j in range(2):
                                kc = 2 * pr + j
                                if kc * KC >= qj * QT:
                                    # mask only the 128-wide diagonal band
                                    o = offs[j]
                                    w = min(KC, QT - o)
                                    nc.gpsimd.affine_select(
                                        out=pt[:, j, o:o + w],
                                        in_=pt[:, j, o:o + w],
                                        compare_op=mybir.AluOpType.is_ge,
                                        fill=0.0, base=qj * QT + o - kc * KC,
                                        pattern=[[1, w]],
                                        channel_multiplier=-1)
                            for j in range(2):
                                kc = 2 * pr + j
                                o = offs[j]
                                nc.tensor.matmul(av[:, o:QT],
                                                 vaug[:, kc, h, :],
                                                 pt[:, j, o:QT],
                                                 start=(kc == 0),
                                                 stop=(kc == nkc - 1))
                        if qj == NQ - 1 and h >= 2:
                            nc.scalar.copy(yTun[:, qj * HPC + h, :], av[:])
                        else:
                            nc.vector.tensor_copy(
                                out=yTun[:, qj * HPC + h, :], in_=av[:])
                        if h == 1:
                            rt_half0 = recip_half(qj, 0)
                    cur = (qj, rt_half0 + recip_half(qj, 1))
                    if prev_qj is not None:
                        normmul(*prev_qj)
                    if qj >= 2:
                        proj_slice(qj - 2)
                    prev_qj = cur
                proj_slice(2)
                normmul(*prev_qj)
                proj_slice(3)
                for p in reversed(attn_scope):
                    p.__exit__(None, None, None)
    return nc


# ---------------------------------------------------------------- host

_NC_CACHE = []


def _get_nc():
    if not _NC_CACHE:
        _install_bir_fix()
        _NC_CACHE.append(build())
    return _NC_CACHE[0]


def make_in_maps(x, w_attn, w_proj):
    in_maps = []
    for c in range(N_CORES):
        b, h0 = c // 4, (c % 4) * HPC
        wq = w_attn[:, h0 * D:(h0 + HPC) * D]
        wk = w_attn[:, E + h0 * D:E + (h0 + HPC) * D]
        wv = w_attn[:, 2 * E + h0 * D:2 * E + (h0 + HPC) * D]
        in_maps.append({
            "xT": np.ascontiguousarray(x[b].T),
            "wqk": np.ascontiguousarray(np.concatenate([wq, wk], axis=1)),
            "wv": np.ascontiguousarray(wv),
            "wproj": np.ascontiguousarray(w_proj[h0 * D:(h0 + HPC) * D, :]),
        })
    return in_maps


def run(x, w_attn, w_proj, trace=False, tmpdir=None):
    from concourse.bass_utils import run_bass_kernel_spmd
    nc = _get_nc()
    res = run_bass_kernel_spmd(nc, make_in_maps(x, w_attn, w_proj),
                               list(range(N_CORES)), trace=trace, tmpdir=tmpdir)
    y = np.zeros((B, S, E), np.float32)
    for c in range(N_CORES):
        y[c // 4] += res.results[c]["y"]
    return y, res


def kernel(x, w_attn, w_proj):
    y, _ = run(np.asarray(x, np.float32), np.asarray(w_attn, np.float32),
               np.asarray(w_proj, np.float32))
    return y


# revision 21
# speedup vs baseline: 1.0178x; 1.0178x over previous
"""Causal self-attention (B=2, S=2048, E=1024, H=16) on 8 trn2 cores.

Sharding: batch x head -- core c handles batch c//4 and the 4 heads
starting at (c%4)*4. Each core runs QKV projection for its heads,
causal attention, and its slice of the output projection (row-split
c_proj); the host sums the 4 partial projections per batch.

Layout trick: scores are computed transposed (S^T[k, q]) so every
matmul streams N=512 moving columns, and the attention output comes
out as y^T[d, q] -- exactly the stationary operand the output
projection needs. Row-sums ride along as a ones-column appended to V.
All matmul inputs are float32r (~1e-4 rel err, full PE rate at N>=256).
"""

import os
import sys

import numpy as np

_DIR = os.path.dirname(os.path.abspath(__file__))
for _p in (_DIR,):
    if _p not in sys.path:
        sys.path.insert(0, _p)

import concourse.bass as bass
import concourse.mybir as mybir
from concourse import tile
from concourse.vector_clock import ScopedClock, VectorClock

F32 = mybir.dt.float32
F32R = mybir.dt.float32r

B, S, E, H, D = 2, 2048, 1024, 16, 64
HPC = 4          # heads per core
N_CORES = 8
QT = 512         # q tile (moving dim)
KC = 128         # k chunk (contraction tile)


class SplitDrainTileContext(tile.TileContext):
    """Kernel-tail drain with its sem waits split one per instruction.

    The walrus build here rejects instructions carrying more sync waits
    than their ISA struct encodes; TileContext hangs one wait per live
    proc on a single Drain. Sequential single-wait drains on the sync
    engine give the same guarantee.
    """

    def _drain_and_barrier(self, tick_clock, wait_clock):
        gc = list(tick_clock.global_clock)
        n = len(gc)
        for i, t in enumerate(gc):
            if t:
                vc = VectorClock([t if j == i else 0 for j in range(n)])
                inst = self.nc.sync.drain()
                wait_clock.add_sem_waits(inst.ins, ScopedClock({None: vc}))
        self.nc.all_engine_barrier()
        assert self.sems is not None
        popped = self.nc._tile_sem_poison_stack.pop()
        assert popped is self._sem_poison
        self.nc.clear_and_free_semaphores(list(self.sems.allocated().values()))
        self.nc.all_engine_barrier()


# ---------------------------------------------------------------- BIR fix

_CAPS = {"EventSemaphore": 2}
_DEFAULT_CAP = 1
_counter = [0]


def _split_bir_waits(bir):
    """Move excess sync waits onto EventSemaphores inserted just before
    the overloaded instruction (same engine => same program order)."""
    n = 0
    for fn in bir.get("functions", []):
        for bb in fn.get("blocks", []):
            out = []
            for inst in bb.get("instructions", []):
                si = inst.get("sync_info")
                waits = si.get("on_wait") if si else None
                cap = _CAPS.get(inst.get("opcode"), _DEFAULT_CAP)
                if waits and len(waits) > cap:
                    excess, keep = waits[:-cap], waits[-cap:]
                    for i in range(0, len(excess), 2):
                        _counter[0] += 1
                        out.append({
                            "debug": inst.get("debug", 0),
                            "engine": inst["engine"],
                            "ins": [], "outs": [],
                            "name": f"antsplitw-{_counter[0]}",
                            "opcode": "EventSemaphore",
                            "sync_info": {"on_update": [],
                                          "on_wait": excess[i:i + 2]},
                        })
                        n += 1
                    si["on_wait"] = keep
                out.append(inst)
            bb["instructions"] = out
    return n


def _install_bir_fix():
    import json
    import concourse.bass2jax as bass2jax
    from concourse.bass_utils import compile_bir_kernel as orig
    if getattr(bass2jax.compile_bir_kernel, "_ant_split", False):
        return

    def wrapped(ant_bir_str, *args, **kwargs):
        bir = json.loads(ant_bir_str)
        if _split_bir_waits(bir):
            ant_bir_str = json.dumps(bir).encode()
        return orig(ant_bir_str, *args, **kwargs)

    wrapped._ant_split = True
    bass2jax.compile_bir_kernel = wrapped


# ---------------------------------------------------------------- device

def build():
    nc = bass.Bass("TRN2", target_bir_lowering=False, debug=False)
    xT_d = nc.dram_tensor("xT", [E, S], F32R, kind="ExternalInput").ap()
    wqk_d = nc.dram_tensor("wqk", [E, 2 * HPC * D], F32R, kind="ExternalInput").ap()
    wv_d = nc.dram_tensor("wv", [E, HPC * D], F32R, kind="ExternalInput").ap()
    wp_d = nc.dram_tensor("wproj", [HPC * D, E], F32R, kind="ExternalInput").ap()
    y_d = nc.dram_tensor("y", [S, E], F32, kind="ExternalOutput").ap()

    EC = E // 128            # 8 contraction chunks over the embedding dim
    NQ = S // QT             # 4 q tiles
    NST = S // 128           # 16 s tiles of 128

    with SplitDrainTileContext(nc) as tc:
        with tc.tile_pool(name="persist", bufs=1) as persist:
            qT_sb = persist.tile([128, 2, S], F32R)    # heads 01 | 23 stacked
            kTpad = persist.tile([128, HPC, S], F32R)  # per head, rows 64+ zero
            vaug = persist.tile([128, NST, HPC, D + 1], F32R)
            yT = persist.tile([128, 2, S], F32R)       # normalized, proj lhsT
            wp_sb = persist.tile([128, 2, E], F32R)
            ones = persist.tile([128, 64], F32R)
            nc.vector.memset(ones[:].bitcast(F32), 1.0)
            nc.vector.memset(vaug[:, :, :, D:D + 1].bitcast(F32), 1.0)
            for h in range(HPC):
                dead = slice(64, 128) if h % 2 == 0 else slice(0, 64)
                nc.vector.memset(kTpad[dead, h, :].bitcast(F32), 0.0)

            # ---- phase 1: qkv projection (scoped input pool) ----
            with (
                tc.tile_pool(name="qkvin", bufs=1) as qkvin,
                tc.tile_pool(name="psq", bufs=4, space="PSUM") as psq,
                tc.tile_pool(name="psv", bufs=2, space="PSUM") as psv,
            ):
                xT_sb = qkvin.tile([128, EC, S], F32R)
                wqk_sb = qkvin.tile([128, EC, 512], F32R)
                wv_sb = qkvin.tile([128, EC, 256], F32R)
                # xT pieces stream column-block-major so the matmul emission
                # order below never head-of-line-blocks on a late chunk;
                # queues split so first-needed data has a dedicated engine
                def xt_piece(eng, ec, q4):
                    eng.dma_start(xT_sb[:, ec, q4 * 512:(q4 + 1) * 512],
                                  xT_d[ec * 128:(ec + 1) * 128, q4 * 512:(q4 + 1) * 512])
                for ec in range(EC):
                    nc.sync.dma_start(wqk_sb[:, ec, :], wqk_d[ec * 128:(ec + 1) * 128, :])
                    xt_piece(nc.scalar, ec, 0)
                for ec in range(EC):
                    nc.gpsimd.dma_start(wv_sb[:, ec, :], wv_d[ec * 128:(ec + 1) * 128, :])
                    xt_piece(nc.sync, ec, 1)
                for ec in range(EC):
                    xt_piece(nc.sync, ec, 2)
                    xt_piece(nc.gpsimd, ec, 3)
                for ci in range(2):
                    nc.scalar.dma_start(wp_sb[:, ci, :], wp_d[ci * 128:(ci + 1) * 128, :])

                def v_groups(q4):
                    # v natural: stationary = xT s-block, moving = wv
                    for st2 in range(4 * q4, 4 * q4 + 4):
                        ps = psv.tile([128, 256], F32)
                        for ec in range(EC):
                            nc.tensor.matmul(
                                ps[:],
                                xT_sb[:, ec, st2 * 128:(st2 + 1) * 128],
                                wv_sb[:, ec, :],
                                start=(ec == 0), stop=(ec == EC - 1))
                        nc.vector.tensor_copy(
                            out=vaug[:, st2, :, 0:D],
                            in_=ps[:, :].rearrange("p (h d) -> p h d", h=HPC))

                for q4 in range(4):
                    # q/k transposed: stationary = w column block, moving = xT
                    st = q4
                    sslc = slice(st * QT, (st + 1) * QT)
                    for rt in range(4):
                        ps = psq.tile([128, QT], F32)
                        for ec in range(EC):
                            nc.tensor.matmul(
                                ps[:],
                                wqk_sb[:, ec, rt * 128:(rt + 1) * 128],
                                xT_sb[:, ec, st * QT:(st + 1) * QT],
                                start=(ec == 0), stop=(ec == EC - 1))
                        if rt < 2:
                            nc.scalar.copy(qT_sb[:, rt, sslc], ps[:])
                        else:
                            # split the head pair into zero-padded per-head k,
                            # each head keeping its q's partition rows
                            nc.scalar.copy(kTpad[0:64, 2 * (rt - 2), sslc],
                                           ps[0:64, :])
                            nc.vector.tensor_copy(
                                out=kTpad[64:128, 2 * (rt - 2) + 1, sslc],
                                in_=ps[64:128, :])
                    if q4 > 0:
                        v_groups(q4 - 1)
                v_groups(3)

            # ---- phase 2: causal attention, transposed, unnormalized ----
            with tc.tile_pool(name="attw", bufs=1) as attw:
                # unnormalized y^T plus rowsums (row 64), one [65,512] slab
                # per (qj, h)
                yTun = attw.tile([65, NQ * HPC, QT], F32)
                attn_scope = (
                    tc.tile_pool(name="ptp", bufs=6),
                    tc.tile_pool(name="nrm", bufs=4),
                    tc.tile_pool(name="rts", bufs=8),
                    tc.tile_pool(name="bcs2", bufs=2),
                    tc.tile_pool(name="pout", bufs=3),
                    tc.tile_pool(name="pss", bufs=2, space="PSUM"),
                    tc.tile_pool(name="psav", bufs=2, space="PSUM"),
                    tc.tile_pool(name="psb", bufs=2, space="PSUM"),
                )
                (ptp, nrm, rts, bcs2, pout, pss, psav, psb) = (
                    p.__enter__() for p in attn_scope)
                ptp = ptp  # generator force
                attn_scope_entered = True

                def recip_half(qj, half):
                    # 1/rowsum for one head pair, started as soon as that
                    # pair's attention blocks close
                    t0 = qj * HPC + 2 * half
                    rs2 = nrm.tile([2, QT], F32, tag="rs2")
                    nc.sync.dma_start(rs2[:, :], yTun[64:65, t0:t0 + 2, :])
                    lg = nrm.tile([2, QT], F32, tag="lg")
                    nc.scalar.activation(lg[:, :], rs2[:, :],
                                         mybir.ActivationFunctionType.Ln)
                    rt2 = nrm.tile([2, QT], F32R, tag="rt2")
                    # exp(-ln(x)) = 1/x; Ln and Exp share one ACT table set,
                    # and ACT is idle at block tails (DVE is not)
                    nc.scalar.activation(rt2[:, :], lg[:, :],
                                         mybir.ActivationFunctionType.Exp,
                                         scale=-1.0)
                    rt_ts = []
                    for i in range(2):
                        rt_t = rts.tile([1, QT], F32R)
                        nc.sync.dma_start(rt_t[:, :], rt2[i:i + 1, :])
                        rt_ts.append(rt_t)
                    return rt_ts

                def normmul(qj, rt_ts):
                    # normalize y^T for this q tile
                    qslc = slice(qj * QT, (qj + 1) * QT)
                    for h in range(HPC):
                        t = qj * HPC + h
                        rt_t = rt_ts[h]
                        bc = psb.tile([64, QT], F32, tag="pb")
                        nc.tensor.matmul(bc[:], ones[0:1, 0:64], rt_t[:, :],
                                         start=True, stop=True)
                        bc_sb = bcs2.tile([64, QT], F32)
                        nc.vector.tensor_copy(out=bc_sb[:], in_=bc[:])
                        po = 64 * (h % 2)
                        with nc.allow_low_precision(reason="proj lhsT"):
                            nc.vector.tensor_mul(yT[po:po + 64, h // 2, qslc],
                                                 yTun[0:64, t, :], bc_sb[:])

                def proj_slice(qj):
                    for qt in range(qj * 4, (qj + 1) * 4):
                        for eo in range(E // 512):
                            pp = psb.tile([128, 512], F32, tag="pb")
                            for ci in range(2):
                                nc.tensor.matmul(
                                    pp[:],
                                    yT[:, ci, qt * 128:(qt + 1) * 128],
                                    wp_sb[:, ci, eo * 512:(eo + 1) * 512],
                                    start=(ci == 0), stop=(ci == 1))
                            po_t = pout.tile([128, 512], F32)
                            nc.vector.tensor_copy(out=po_t[:], in_=pp[:])
                            nc.sync.dma_start(
                                y_d[qt * 128:(qt + 1) * 128,
                                    eo * 512:(eo + 1) * 512],
                                po_t[:])

                prev_qj = None
                for qj in range(NQ):
                    for h in range(HPC):
                        qslc = slice(qj * QT, (qj + 1) * QT)
                        qT_ap = qT_sb[:, h // 2, qslc]
                        nkc = (qj + 1) * QT // KC
                        av = psav.tile([65, QT], F32)
                        for pr in range(nkc // 2):
                            ps = pss.tile([128, 2, QT], F32)
                            offs = [max(0, (2 * pr + j) * KC - qj * QT)
                                    for j in range(2)]
                            for j in range(2):
                                kc = 2 * pr + j
                                o = offs[j]
                                nc.tensor.matmul(
                                    ps[:, j, o:QT],
                                    kTpad[:, h, kc * KC:(kc + 1) * KC],
                                    qT_ap[:, o:QT],
                                    start=True, stop=True)
                            pt = ptp.tile([128, 2, QT], F32R)
                            if offs[0] == 0 and offs[1] == 0:
                                nc.scalar.activation(
                                    pt[:], ps[:],
                                    mybir.ActivationFunctionType.Exp,
                                    scale=0.125)
                            else:
                                # diagonal pair: exp only live columns
                                for j, o in enumerate(offs):
                                    nc.scalar.activation(
                                        pt[:, j, o:QT], ps[:, j, o:QT],
                                        mybir.ActivationFunctionType.Exp,
                                        scale=0.125)
                            for j in range(2):
                                kc = 2 * pr + j
                                if kc * KC >= qj * QT:
                                    # mask only the 128-wide diagonal band
                                    o = offs[j]
                                    w = min(KC, QT - o)
                                    nc.gpsimd.affine_select(
                                        out=pt[:, j, o:o + w],
                                        in_=pt[:, j, o:o + w],
                                        compare_op=mybir.AluOpType.is_ge,
                                        fill=0.0, base=qj * QT + o - kc * KC,
                                        pattern=[[1, w]],
                                        channel_multiplier=-1)
                            for j in range(2):
                                kc = 2 * pr + j
                                o = offs[j]
                                nc.tensor.matmul(av[:, o:QT],
                                                 vaug[:, kc, h, :],
                                                 pt[:, j, o:QT],
                                                 start=(kc == 0),
                                                 stop=(kc == nkc - 1))
                        nc.vector.tensor_copy(
                            out=yTun[:, qj * HPC + h, :], in_=av[:])
                        if h == 1:
                            rt_half0 = recip_half(qj, 0)
                    cur = (qj, rt_half0 + recip_half(qj, 1))
                    if prev_qj is not None:
                        normmul(*prev_qj)
                    if qj >= 2:
                        proj_slice(qj - 2)
                    prev_qj = cur
                proj_slice(2)
                normmul(*prev_qj)
                proj_slice(3)
                for p in reversed(attn_scope):
                    p.__exit__(None, None, None)
    return nc


# ---------------------------------------------------------------- host

_NC_CACHE = []


def _get_nc():
    if not _NC_CACHE:
        _install_bir_fix()
        _NC_CACHE.append(build())
    return _NC_CACHE[0]


def make_in_maps(x, w_attn, w_proj):
    in_maps = []
    for c in range(N_CORES):
        b, h0 = c // 4, (c % 4) * HPC
        wq = w_attn[:, h0 * D:(h0 + HPC) * D]
        wk = w_attn[:, E + h0 * D:E + (h0 + HPC) * D]
        wv = w_attn[:, 2 * E + h0 * D:2 * E + (h0 + HPC) * D]
        in_maps.append({
            "xT": np.ascontiguousarray(x[b].T),
            "wqk": np.ascontiguousarray(np.concatenate([wq, wk], axis=1)),
            "wv": np.ascontiguousarray(wv),
            "wproj": np.ascontiguousarray(w_proj[h0 * D:(h0 + HPC) * D, :]),
        })
    return in_maps


def run(x, w_attn, w_proj, trace=False, tmpdir=None):
    from concourse.bass_utils import run_bass_kernel_spmd
    nc = _get_nc()
    res = run_bass_kernel_spmd(nc, make_in_maps(x, w_attn, w_proj),
                               list(range(N_CORES)), trace=trace, tmpdir=tmpdir)
    y = np.zeros((B, S, E), np.float32)
    for c in range(N_CORES):
        y[c // 4] += res.results[c]["y"]
    return y, res


def kernel(x, w_attn, w_proj):
    y, _ = run(np.asarray(x, np.float32), np.asarray(w_attn, np.float32),
               np.asarray(w_proj, np.float32))
    return y


# revision 22
# speedup vs baseline: 1.0384x; 1.0202x over previous
"""Causal self-attention (B=2, S=2048, E=1024, H=16) on 8 trn2 cores.

Sharding: batch x head -- core c handles batch c//4 and the 4 heads
starting at (c%4)*4. Each core runs QKV projection for its heads,
causal attention, and its slice of the output projection (row-split
c_proj); the host sums the 4 partial projections per batch.

Layout trick: scores are computed transposed (S^T[k, q]) so every
matmul streams N=512 moving columns, and the attention output comes
out as y^T[d, q] -- exactly the stationary operand the output
projection needs. Row-sums ride along as a ones-column appended to V.
All matmul inputs are float32r (~1e-4 rel err, full PE rate at N>=256).
"""

import os
import sys

import numpy as np

_DIR = os.path.dirname(os.path.abspath(__file__))
for _p in (_DIR,):
    if _p not in sys.path:
        sys.path.insert(0, _p)

import concourse.bass as bass
import concourse.mybir as mybir
from concourse import tile
from concourse.vector_clock import ScopedClock, VectorClock

F32 = mybir.dt.float32
F32R = mybir.dt.float32r
F16 = mybir.dt.float16
U16 = mybir.dt.uint16

B, S, E, H, D = 2, 2048, 1024, 16, 64
HPC = 4          # heads per core
N_CORES = 8
QT = 512         # q tile (moving dim)
KC = 128         # k chunk (contraction tile)


class SplitDrainTileContext(tile.TileContext):
    """Kernel-tail drain with its sem waits split one per instruction.

    The walrus build here rejects instructions carrying more sync waits
    than their ISA struct encodes; TileContext hangs one wait per live
    proc on a single Drain. Sequential single-wait drains on the sync
    engine give the same guarantee.
    """

    def _drain_and_barrier(self, tick_clock, wait_clock):
        gc = list(tick_clock.global_clock)
        n = len(gc)
        for i, t in enumerate(gc):
            if t:
                vc = VectorClock([t if j == i else 0 for j in range(n)])
                inst = self.nc.sync.drain()
                wait_clock.add_sem_waits(inst.ins, ScopedClock({None: vc}))
        self.nc.all_engine_barrier()
        assert self.sems is not None
        popped = self.nc._tile_sem_poison_stack.pop()
        assert popped is self._sem_poison
        self.nc.clear_and_free_semaphores(list(self.sems.allocated().values()))
        self.nc.all_engine_barrier()


# ---------------------------------------------------------------- BIR fix

_CAPS = {"EventSemaphore": 2}
_DEFAULT_CAP = 1
_counter = [0]


def _split_bir_waits(bir):
    """Move excess sync waits onto EventSemaphores inserted just before
    the overloaded instruction (same engine => same program order)."""
    n = 0
    for fn in bir.get("functions", []):
        for bb in fn.get("blocks", []):
            out = []
            for inst in bb.get("instructions", []):
                si = inst.get("sync_info")
                waits = si.get("on_wait") if si else None
                cap = _CAPS.get(inst.get("opcode"), _DEFAULT_CAP)
                if waits and len(waits) > cap:
                    excess, keep = waits[:-cap], waits[-cap:]
                    for i in range(0, len(excess), 2):
                        _counter[0] += 1
                        out.append({
                            "debug": inst.get("debug", 0),
                            "engine": inst["engine"],
                            "ins": [], "outs": [],
                            "name": f"antsplitw-{_counter[0]}",
                            "opcode": "EventSemaphore",
                            "sync_info": {"on_update": [],
                                          "on_wait": excess[i:i + 2]},
                        })
                        n += 1
                    si["on_wait"] = keep
                out.append(inst)
            bb["instructions"] = out
    return n


def _install_bir_fix():
    import json
    import concourse.bass2jax as bass2jax
    from concourse.bass_utils import compile_bir_kernel as orig
    if getattr(bass2jax.compile_bir_kernel, "_ant_split", False):
        return

    def wrapped(ant_bir_str, *args, **kwargs):
        bir = json.loads(ant_bir_str)
        if _split_bir_waits(bir):
            ant_bir_str = json.dumps(bir).encode()
        return orig(ant_bir_str, *args, **kwargs)

    wrapped._ant_split = True
    bass2jax.compile_bir_kernel = wrapped


# ---------------------------------------------------------------- device

def build():
    nc = bass.Bass("TRN2", target_bir_lowering=False, debug=False)
    xT_d = nc.dram_tensor("xT", [E, S], F32R, kind="ExternalInput").ap()
    wqk_d = nc.dram_tensor("wqk", [E, 2 * HPC * D], F32R, kind="ExternalInput").ap()
    wv_d = nc.dram_tensor("wv", [E, HPC * D], F32R, kind="ExternalInput").ap()
    wp_d = nc.dram_tensor("wproj", [HPC * D, E], F32R, kind="ExternalInput").ap()
    y_d = nc.dram_tensor("y", [S, E], F32, kind="ExternalOutput").ap()

    EC = E // 128            # 8 contraction chunks over the embedding dim
    NQ = S // QT             # 4 q tiles
    NST = S // 128           # 16 s tiles of 128

    with SplitDrainTileContext(nc) as tc:
        with tc.tile_pool(name="persist", bufs=1) as persist:
            qT_sb = persist.tile([128, 2, S], F16)    # heads 01 | 23 stacked
            kTpad = persist.tile([128, HPC, S], F16)  # per head, rows 64+ zero
            vaug = persist.tile([128, NST, HPC, D + 1], F16)
            yT = persist.tile([128, 2, S], F32R)       # normalized, proj lhsT
            wp_sb = persist.tile([128, 2, E], F32R)
            ones = persist.tile([128, 64], F32R)
            nc.vector.memset(ones[:].bitcast(F32), 1.0)
            nc.vector.memset(vaug[:, :, :, D:D + 1].bitcast(U16), 15360)  # fp16 1.0
            for h in range(HPC):
                dead = slice(64, 128) if h % 2 == 0 else slice(0, 64)
                nc.vector.memset(kTpad[dead, h, :].bitcast(U16), 0)

            # ---- phase 1: qkv projection (scoped input pool) ----
            with (
                tc.tile_pool(name="qkvin", bufs=1) as qkvin,
                tc.tile_pool(name="psq", bufs=4, space="PSUM") as psq,
                tc.tile_pool(name="psv", bufs=2, space="PSUM") as psv,
            ):
                xT_sb = qkvin.tile([128, EC, S], F32R)
                wqk_sb = qkvin.tile([128, EC, 512], F32R)
                wv_sb = qkvin.tile([128, EC, 256], F32R)
                # xT pieces stream column-block-major so the matmul emission
                # order below never head-of-line-blocks on a late chunk;
                # queues split so first-needed data has a dedicated engine
                def xt_piece(eng, ec, q4):
                    eng.dma_start(xT_sb[:, ec, q4 * 512:(q4 + 1) * 512],
                                  xT_d[ec * 128:(ec + 1) * 128, q4 * 512:(q4 + 1) * 512])
                for ec in range(EC):
                    nc.sync.dma_start(wqk_sb[:, ec, :], wqk_d[ec * 128:(ec + 1) * 128, :])
                    xt_piece(nc.scalar, ec, 0)
                for ec in range(EC):
                    nc.gpsimd.dma_start(wv_sb[:, ec, :], wv_d[ec * 128:(ec + 1) * 128, :])
                    xt_piece(nc.sync, ec, 1)
                for ec in range(EC):
                    xt_piece(nc.sync, ec, 2)
                    xt_piece(nc.gpsimd, ec, 3)
                for ci in range(2):
                    nc.scalar.dma_start(wp_sb[:, ci, :], wp_d[ci * 128:(ci + 1) * 128, :])

                def v_groups(q4):
                    # v natural: stationary = xT s-block, moving = wv
                    for st2 in range(4 * q4, 4 * q4 + 4):
                        ps = psv.tile([128, 256], F32)
                        for ec in range(EC):
                            nc.tensor.matmul(
                                ps[:],
                                xT_sb[:, ec, st2 * 128:(st2 + 1) * 128],
                                wv_sb[:, ec, :],
                                start=(ec == 0), stop=(ec == EC - 1))
                        nc.vector.tensor_copy(
                            out=vaug[:, st2, :, 0:D],
                            in_=ps[:, :].rearrange("p (h d) -> p h d", h=HPC))

                for q4 in range(4):
                    # q/k transposed: stationary = w column block, moving = xT
                    st = q4
                    sslc = slice(st * QT, (st + 1) * QT)
                    for rt in range(4):
                        ps = psq.tile([128, QT], F32)
                        for ec in range(EC):
                            nc.tensor.matmul(
                                ps[:],
                                wqk_sb[:, ec, rt * 128:(rt + 1) * 128],
                                xT_sb[:, ec, st * QT:(st + 1) * QT],
                                start=(ec == 0), stop=(ec == EC - 1))
                        if rt < 2:
                            nc.scalar.copy(qT_sb[:, rt, sslc], ps[:])
                        else:
                            # split the head pair into zero-padded per-head k,
                            # each head keeping its q's partition rows
                            nc.scalar.copy(kTpad[0:64, 2 * (rt - 2), sslc],
                                           ps[0:64, :])
                            nc.vector.tensor_copy(
                                out=kTpad[64:128, 2 * (rt - 2) + 1, sslc],
                                in_=ps[64:128, :])
                    if q4 > 0:
                        v_groups(q4 - 1)
                v_groups(3)

            # ---- phase 2: causal attention, transposed, unnormalized ----
            with tc.tile_pool(name="attw", bufs=1) as attw:
                # unnormalized y^T plus rowsums (row 64), one [65,512] slab
                # per (qj, h)
                yTun = attw.tile([65, NQ * HPC, QT], F32)
                attn_scope = (
                    tc.tile_pool(name="ptp", bufs=6),
                    tc.tile_pool(name="nrm", bufs=4),
                    tc.tile_pool(name="rts", bufs=8),
                    tc.tile_pool(name="bcs2", bufs=2),
                    tc.tile_pool(name="pout", bufs=3),
                    tc.tile_pool(name="pss", bufs=2, space="PSUM"),
                    tc.tile_pool(name="psav", bufs=2, space="PSUM"),
                    tc.tile_pool(name="psb", bufs=2, space="PSUM"),
                )
                (ptp, nrm, rts, bcs2, pout, pss, psav, psb) = (
                    p.__enter__() for p in attn_scope)
                ptp = ptp  # generator force
                attn_scope_entered = True

                def recip_half(qj, half):
                    # 1/rowsum for one head pair, started as soon as that
                    # pair's attention blocks close
                    t0 = qj * HPC + 2 * half
                    rs2 = nrm.tile([2, QT], F32, tag="rs2")
                    nc.sync.dma_start(rs2[:, :], yTun[64:65, t0:t0 + 2, :])
                    lg = nrm.tile([2, QT], F32, tag="lg")
                    nc.scalar.activation(lg[:, :], rs2[:, :],
                                         mybir.ActivationFunctionType.Ln)
                    rt2 = nrm.tile([2, QT], F32R, tag="rt2")
                    # exp(-ln(x)) = 1/x; Ln and Exp share one ACT table set,
                    # and ACT is idle at block tails (DVE is not)
                    nc.scalar.activation(rt2[:, :], lg[:, :],
                                         mybir.ActivationFunctionType.Exp,
                                         scale=-1.0)
                    rt_ts = []
                    for i in range(2):
                        rt_t = rts.tile([1, QT], F32R)
                        nc.sync.dma_start(rt_t[:, :], rt2[i:i + 1, :])
                        rt_ts.append(rt_t)
                    return rt_ts

                def normmul(qj, rt_ts):
                    # normalize y^T for this q tile
                    qslc = slice(qj * QT, (qj + 1) * QT)
                    for h in range(HPC):
                        t = qj * HPC + h
                        rt_t = rt_ts[h]
                        bc = psb.tile([64, QT], F32, tag="pb")
                        nc.tensor.matmul(bc[:], ones[0:1, 0:64], rt_t[:, :],
                                         start=True, stop=True)
                        bc_sb = bcs2.tile([64, QT], F32)
                        nc.vector.tensor_copy(out=bc_sb[:], in_=bc[:])
                        po = 64 * (h % 2)
                        with nc.allow_low_precision(reason="proj lhsT"):
                            nc.vector.tensor_mul(yT[po:po + 64, h // 2, qslc],
                                                 yTun[0:64, t, :], bc_sb[:])

                def proj_slice(qj):
                    for qt in range(qj * 4, (qj + 1) * 4):
                        for eo in range(E // 512):
                            pp = psb.tile([128, 512], F32, tag="pb")
                            for ci in range(2):
                                nc.tensor.matmul(
                                    pp[:],
                                    yT[:, ci, qt * 128:(qt + 1) * 128],
                                    wp_sb[:, ci, eo * 512:(eo + 1) * 512],
                                    start=(ci == 0), stop=(ci == 1))
                            po_t = pout.tile([128, 512], F32)
                            nc.vector.tensor_copy(out=po_t[:], in_=pp[:])
                            nc.sync.dma_start(
                                y_d[qt * 128:(qt + 1) * 128,
                                    eo * 512:(eo + 1) * 512],
                                po_t[:])

                prev_qj = None
                for qj in range(NQ):
                    for h in range(HPC):
                        qslc = slice(qj * QT, (qj + 1) * QT)
                        qT_ap = qT_sb[:, h // 2, qslc]
                        nkc = (qj + 1) * QT // KC
                        av = psav.tile([65, QT], F32)
                        for pr in range(nkc // 2):
                            ps = pss.tile([128, 2, QT], F32)
                            offs = [max(0, (2 * pr + j) * KC - qj * QT)
                                    for j in range(2)]
                            for j in range(2):
                                kc = 2 * pr + j
                                o = offs[j]
                                nc.tensor.matmul(
                                    ps[:, j, o:QT],
                                    kTpad[:, h, kc * KC:(kc + 1) * KC],
                                    qT_ap[:, o:QT],
                                    start=True, stop=True)
                            pt = ptp.tile([128, 2, QT], F16)
                            if offs[0] == 0 and offs[1] == 0:
                                nc.scalar.activation(
                                    pt[:], ps[:],
                                    mybir.ActivationFunctionType.Exp,
                                    scale=0.125)
                            else:
                                # diagonal pair: exp only live columns
                                for j, o in enumerate(offs):
                                    nc.scalar.activation(
                                        pt[:, j, o:QT], ps[:, j, o:QT],
                                        mybir.ActivationFunctionType.Exp,
                                        scale=0.125)
                            for j in range(2):
                                kc = 2 * pr + j
                                if kc * KC >= qj * QT:
                                    # mask only the 128-wide diagonal band
                                    o = offs[j]
                                    w = min(KC, QT - o)
                                    nc.gpsimd.affine_select(
                                        out=pt[:, j, o:o + w],
                                        in_=pt[:, j, o:o + w],
                                        compare_op=mybir.AluOpType.is_ge,
                                        fill=0.0, base=qj * QT + o - kc * KC,
                                        pattern=[[1, w]],
                                        channel_multiplier=-1)
                            for j in range(2):
                                kc = 2 * pr + j
                                o = offs[j]
                                nc.tensor.matmul(av[:, o:QT],
                                                 vaug[:, kc, h, :],
                                                 pt[:, j, o:QT],
                                                 start=(kc == 0),
                                                 stop=(kc == nkc - 1))
                        nc.vector.tensor_copy(
                            out=yTun[:, qj * HPC + h, :], in_=av[:])
                        if h == 1:
                            rt_half0 = recip_half(qj, 0)
                    cur = (qj, rt_half0 + recip_half(qj, 1))
                    if prev_qj is not None:
                        normmul(*prev_qj)
                    if qj >= 2:
                        proj_slice(qj - 2)
                    prev_qj = cur
                proj_slice(2)
                normmul(*prev_qj)
                proj_slice(3)
                for p in reversed(attn_scope):
                    p.__exit__(None, None, None)
    return nc


# ---------------------------------------------------------------- host

_NC_CACHE = []


def _get_nc():
    if not _NC_CACHE:
        _install_bir_fix()
        _NC_CACHE.append(build())
    return _NC_CACHE[0]


def make_in_maps(x, w_attn, w_proj):
    in_maps = []
    for c in range(N_CORES):
        b, h0 = c // 4, (c % 4) * HPC
        wq = w_attn[:, h0 * D:(h0 + HPC) * D]
        wk = w_attn[:, E + h0 * D:E + (h0 + HPC) * D]
        wv = w_attn[:, 2 * E + h0 * D:2 * E + (h0 + HPC) * D]
        in_maps.append({
            "xT": np.ascontiguousarray(x[b].T),
            "wqk": np.ascontiguousarray(np.concatenate([wq, wk], axis=1)),
            "wv": np.ascontiguousarray(wv),
            "wproj": np.ascontiguousarray(w_proj[h0 * D:(h0 + HPC) * D, :]),
        })
    return in_maps


def run(x, w_attn, w_proj, trace=False, tmpdir=None):
    from concourse.bass_utils import run_bass_kernel_spmd
    nc = _get_nc()
    res = run_bass_kernel_spmd(nc, make_in_maps(x, w_attn, w_proj),
                               list(range(N_CORES)), trace=trace, tmpdir=tmpdir)
    y = np.zeros((B, S, E), np.float32)
    for c in range(N_CORES):
        y[c // 4] += res.results[c]["y"]
    return y, res


def kernel(x, w_attn, w_proj):
    y, _ = run(np.asarray(x, np.float32), np.asarray(w_attn, np.float32),
               np.asarray(w_proj, np.float32))
    return y


# revision 24
# speedup vs baseline: 1.0539x; 1.0150x over previous
"""Causal self-attention (B=2, S=2048, E=1024, H=16) on 8 trn2 cores.

Sharding: batch x head -- core c handles batch c//4 and the 4 heads
starting at (c%4)*4. Each core runs QKV projection for its heads,
causal attention, and its slice of the output projection (row-split
c_proj); the host sums the 4 partial projections per batch.

Layout trick: scores are computed transposed (S^T[k, q]) so every
matmul streams N=512 moving columns, and the attention output comes
out as y^T[d, q] -- exactly the stationary operand the output
projection needs. Row-sums ride along as a ones-column appended to V.
All matmul inputs are float32r (~1e-4 rel err, full PE rate at N>=256).
"""

import os
import sys

import numpy as np

_DIR = os.path.dirname(os.path.abspath(__file__))
for _p in (_DIR,):
    if _p not in sys.path:
        sys.path.insert(0, _p)

import concourse.bass as bass
import concourse.mybir as mybir
from concourse import tile
from concourse.vector_clock import ScopedClock, VectorClock

F32 = mybir.dt.float32
F32R = mybir.dt.float32r
F16 = mybir.dt.float16
U16 = mybir.dt.uint16

B, S, E, H, D = 2, 2048, 1024, 16, 64
HPC = 4          # heads per core
N_CORES = 8
QT = 512         # q tile (moving dim)
KC = 128         # k chunk (contraction tile)


class SplitDrainTileContext(tile.TileContext):
    """Kernel-tail drain with its sem waits split one per instruction.

    The walrus build here rejects instructions carrying more sync waits
    than their ISA struct encodes; TileContext hangs one wait per live
    proc on a single Drain. Sequential single-wait drains on the sync
    engine give the same guarantee.
    """

    def _drain_and_barrier(self, tick_clock, wait_clock):
        gc = list(tick_clock.global_clock)
        n = len(gc)
        for i, t in enumerate(gc):
            if t:
                vc = VectorClock([t if j == i else 0 for j in range(n)])
                inst = self.nc.sync.drain()
                wait_clock.add_sem_waits(inst.ins, ScopedClock({None: vc}))
        self.nc.all_engine_barrier()
        assert self.sems is not None
        popped = self.nc._tile_sem_poison_stack.pop()
        assert popped is self._sem_poison
        self.nc.clear_and_free_semaphores(list(self.sems.allocated().values()))
        self.nc.all_engine_barrier()


# ---------------------------------------------------------------- BIR fix

_CAPS = {"EventSemaphore": 2}
_DEFAULT_CAP = 1
_counter = [0]


def _split_bir_waits(bir):
    """Move excess sync waits onto EventSemaphores inserted just before
    the overloaded instruction (same engine => same program order)."""
    n = 0
    for fn in bir.get("functions", []):
        for bb in fn.get("blocks", []):
            out = []
            for inst in bb.get("instructions", []):
                si = inst.get("sync_info")
                waits = si.get("on_wait") if si else None
                cap = _CAPS.get(inst.get("opcode"), _DEFAULT_CAP)
                if waits and len(waits) > cap:
                    excess, keep = waits[:-cap], waits[-cap:]
                    for i in range(0, len(excess), 2):
                        _counter[0] += 1
                        out.append({
                            "debug": inst.get("debug", 0),
                            "engine": inst["engine"],
                            "ins": [], "outs": [],
                            "name": f"antsplitw-{_counter[0]}",
                            "opcode": "EventSemaphore",
                            "sync_info": {"on_update": [],
                                          "on_wait": excess[i:i + 2]},
                        })
                        n += 1
                    si["on_wait"] = keep
                out.append(inst)
            bb["instructions"] = out
    return n


def _install_bir_fix():
    import json
    import concourse.bass2jax as bass2jax
    from concourse.bass_utils import compile_bir_kernel as orig
    if getattr(bass2jax.compile_bir_kernel, "_ant_split", False):
        return

    def wrapped(ant_bir_str, *args, **kwargs):
        bir = json.loads(ant_bir_str)
        if _split_bir_waits(bir):
            ant_bir_str = json.dumps(bir).encode()
        return orig(ant_bir_str, *args, **kwargs)

    wrapped._ant_split = True
    bass2jax.compile_bir_kernel = wrapped


# ---------------------------------------------------------------- device

def build():
    nc = bass.Bass("TRN2", target_bir_lowering=False, debug=False)
    xT_d = nc.dram_tensor("xT", [E, S], F32R, kind="ExternalInput").ap()
    wqk_d = nc.dram_tensor("wqk", [E, 2 * HPC * D], F32R, kind="ExternalInput").ap()
    wv_d = nc.dram_tensor("wv", [E, HPC * D], F32R, kind="ExternalInput").ap()
    wp_d = nc.dram_tensor("wproj", [HPC * D, E], F32R, kind="ExternalInput").ap()
    y_d = nc.dram_tensor("y", [S, E], F32, kind="ExternalOutput").ap()

    EC = E // 128            # 8 contraction chunks over the embedding dim
    NQ = S // QT             # 4 q tiles
    NST = S // 128           # 16 s tiles of 128

    with SplitDrainTileContext(nc) as tc:
        with tc.tile_pool(name="persist", bufs=1) as persist:
            qT_sb = persist.tile([128, 2, S], F16)    # heads 01 | 23 stacked
            kTpad = persist.tile([128, HPC, S], F16)  # per head, rows 64+ zero
            vaug = persist.tile([128, NST, HPC, D + 1], F16)
            yT = persist.tile([128, 2, S], F32R)       # normalized, proj lhsT
            wp_sb = persist.tile([128, 2, E], F32R)
            ones = persist.tile([128, 64], F32R)
            nc.vector.memset(ones[:].bitcast(F32), 1.0)
            nc.vector.memset(vaug[:, :, :, D:D + 1].bitcast(U16), 15360)  # fp16 1.0
            for h in range(HPC):
                dead = slice(64, 128) if h % 2 == 0 else slice(0, 64)
                nc.vector.memset(kTpad[dead, h, :].bitcast(U16), 0)

            # ---- phase 1: qkv projection (scoped input pool) ----
            with (
                tc.tile_pool(name="qkvin", bufs=1) as qkvin,
                tc.tile_pool(name="psq", bufs=4, space="PSUM") as psq,
                tc.tile_pool(name="psv", bufs=2, space="PSUM") as psv,
            ):
                xT_sb = qkvin.tile([128, EC, S], F32R)
                wqk_sb = qkvin.tile([128, EC, 512], F32R)
                wv_sb = qkvin.tile([128, EC, 256], F32R)
                # xT pieces stream column-block-major so the matmul emission
                # order below never head-of-line-blocks on a late chunk;
                # queues split so first-needed data has a dedicated engine
                def xt_piece(eng, ec, q4):
                    eng.dma_start(xT_sb[:, ec, q4 * 512:(q4 + 1) * 512],
                                  xT_d[ec * 128:(ec + 1) * 128, q4 * 512:(q4 + 1) * 512])
                for ec in range(EC):
                    nc.sync.dma_start(wqk_sb[:, ec, :], wqk_d[ec * 128:(ec + 1) * 128, :])
                    xt_piece(nc.scalar, ec, 0)
                for ec in range(EC):
                    nc.gpsimd.dma_start(wv_sb[:, ec, :], wv_d[ec * 128:(ec + 1) * 128, :])
                    xt_piece(nc.sync, ec, 1)
                for ec in range(EC):
                    xt_piece(nc.sync, ec, 2)
                    xt_piece(nc.gpsimd, ec, 3)
                for ci in range(2):
                    nc.scalar.dma_start(wp_sb[:, ci, :], wp_d[ci * 128:(ci + 1) * 128, :])

                def v_groups(q4):
                    # v natural: stationary = xT s-block, moving = wv
                    for st2 in range(4 * q4, 4 * q4 + 4):
                        ps = psv.tile([128, 256], F32)
                        for ec in range(EC):
                            nc.tensor.matmul(
                                ps[:],
                                xT_sb[:, ec, st2 * 128:(st2 + 1) * 128],
                                wv_sb[:, ec, :],
                                start=(ec == 0), stop=(ec == EC - 1))
                        nc.vector.tensor_copy(
                            out=vaug[:, st2, :, 0:D],
                            in_=ps[:, :].rearrange("p (h d) -> p h d", h=HPC))

                for q4 in range(4):
                    # q/k transposed: stationary = w column block, moving = xT
                    st = q4
                    sslc = slice(st * QT, (st + 1) * QT)
                    for rt in range(4):
                        ps = psq.tile([128, QT], F32)
                        for ec in range(EC):
                            nc.tensor.matmul(
                                ps[:],
                                wqk_sb[:, ec, rt * 128:(rt + 1) * 128],
                                xT_sb[:, ec, st * QT:(st + 1) * QT],
                                start=(ec == 0), stop=(ec == EC - 1))
                        if rt < 2:
                            nc.scalar.copy(qT_sb[:, rt, sslc], ps[:])
                        else:
                            # split the head pair into zero-padded per-head k,
                            # each head keeping its q's partition rows
                            nc.scalar.copy(kTpad[0:64, 2 * (rt - 2), sslc],
                                           ps[0:64, :])
                            nc.vector.tensor_copy(
                                out=kTpad[64:128, 2 * (rt - 2) + 1, sslc],
                                in_=ps[64:128, :])
                    if q4 > 0:
                        v_groups(q4 - 1)
                v_groups(3)

            # ---- phase 2: causal attention, transposed, unnormalized ----
            with tc.tile_pool(name="attw", bufs=1) as attw:
                # unnormalized y^T plus rowsums (row 64), one [65,512] slab
                # per (qj, h)
                yTun = attw.tile([65, NQ * HPC, QT], F32)
                attn_scope = (
                    tc.tile_pool(name="ptp", bufs=6),
                    tc.tile_pool(name="nrm", bufs=4),
                    tc.tile_pool(name="rts", bufs=8),
                    tc.tile_pool(name="bcs2", bufs=2),
                    tc.tile_pool(name="pout", bufs=3),
                    tc.tile_pool(name="pss", bufs=2, space="PSUM"),
                    tc.tile_pool(name="psav", bufs=2, space="PSUM"),
                    tc.tile_pool(name="psb", bufs=2, space="PSUM"),
                )
                (ptp, nrm, rts, bcs2, pout, pss, psav, psb) = (
                    p.__enter__() for p in attn_scope)
                ptp = ptp  # generator force
                attn_scope_entered = True

                def recip_half(qj, half):
                    # 1/rowsum for one head pair, started as soon as that
                    # pair's attention blocks close
                    t0 = qj * HPC + 2 * half
                    rs2 = nrm.tile([2, QT], F32, tag="rs2")
                    nc.sync.dma_start(rs2[:, :], yTun[64:65, t0:t0 + 2, :])
                    lg = nrm.tile([2, QT], F32, tag="lg")
                    nc.scalar.activation(lg[:, :], rs2[:, :],
                                         mybir.ActivationFunctionType.Ln)
                    rt2 = nrm.tile([2, QT], F32R, tag="rt2")
                    # exp(-ln(x)) = 1/x; Ln and Exp share one ACT table set,
                    # and ACT is idle at block tails (DVE is not)
                    nc.scalar.activation(rt2[:, :], lg[:, :],
                                         mybir.ActivationFunctionType.Exp,
                                         scale=-1.0)
                    rt_ts = []
                    for i in range(2):
                        rt_t = rts.tile([1, QT], F32R)
                        nc.sync.dma_start(rt_t[:, :], rt2[i:i + 1, :])
                        rt_ts.append(rt_t)
                    return rt_ts

                def normmul(qj, rt_ts):
                    # normalize y^T for this q tile
                    qslc = slice(qj * QT, (qj + 1) * QT)
                    for h in range(HPC):
                        t = qj * HPC + h
                        rt_t = rt_ts[h]
                        bc = psb.tile([64, QT], F32, tag="pb")
                        nc.tensor.matmul(bc[:], ones[0:1, 0:64], rt_t[:, :],
                                         start=True, stop=True)
                        bc_sb = bcs2.tile([64, QT], F32)
                        nc.vector.tensor_copy(out=bc_sb[:], in_=bc[:])
                        po = 64 * (h % 2)
                        with nc.allow_low_precision(reason="proj lhsT"):
                            nc.vector.tensor_mul(yT[po:po + 64, h // 2, qslc],
                                                 yTun[0:64, t, :], bc_sb[:])

                def proj_slice(qj):
                    for qt in range(qj * 4, (qj + 1) * 4):
                        for eo in range(E // 512):
                            pp = psb.tile([128, 512], F32, tag="pb")
                            for ci in range(2):
                                nc.tensor.matmul(
                                    pp[:],
                                    yT[:, ci, qt * 128:(qt + 1) * 128],
                                    wp_sb[:, ci, eo * 512:(eo + 1) * 512],
                                    start=(ci == 0), stop=(ci == 1))
                            po_t = pout.tile([128, 512], F32)
                            nc.vector.tensor_copy(out=po_t[:], in_=pp[:])
                            nc.sync.dma_start(
                                y_d[qt * 128:(qt + 1) * 128,
                                    eo * 512:(eo + 1) * 512],
                                po_t[:])

                prev_qj = None
                for qj in range(NQ):
                    for h in range(HPC):
                        qslc = slice(qj * QT, (qj + 1) * QT)
                        qT_ap = qT_sb[:, h // 2, qslc]
                        nkc = (qj + 1) * QT // KC
                        av = psav.tile([65, QT], F32)
                        for pr in range(nkc // 2):
                            ps = pss.tile([128, 2, QT], F32)
                            offs = [max(0, (2 * pr + j) * KC - qj * QT)
                                    for j in range(2)]
                            for j in range(2):
                                kc = 2 * pr + j
                                o = offs[j]
                                nc.tensor.matmul(
                                    ps[:, j, o:QT],
                                    kTpad[:, h, kc * KC:(kc + 1) * KC],
                                    qT_ap[:, o:QT],
                                    start=True, stop=True)
                            pt = ptp.tile([128, 2, QT], F16)
                            if offs[0] == 0 and offs[1] == 0:
                                nc.scalar.activation(
                                    pt[:], ps[:],
                                    mybir.ActivationFunctionType.Exp,
                                    scale=0.125)
                            else:
                                # diagonal pair: exp only live columns
                                for j, o in enumerate(offs):
                                    nc.scalar.activation(
                                        pt[:, j, o:QT], ps[:, j, o:QT],
                                        mybir.ActivationFunctionType.Exp,
                                        scale=0.125)
                            for j in range(2):
                                kc = 2 * pr + j
                                if kc * KC >= qj * QT:
                                    # mask only the 128-wide diagonal band
                                    o = offs[j]
                                    w = min(KC, QT - o)
                                    nc.gpsimd.affine_select(
                                        out=pt[:, j, o:o + w],
                                        in_=pt[:, j, o:o + w],
                                        compare_op=mybir.AluOpType.is_ge,
                                        fill=0.0, base=qj * QT + o - kc * KC,
                                        pattern=[[1, w]],
                                        channel_multiplier=-1)
                            for j in range(2):
                                kc = 2 * pr + j
                                o = offs[j]
                                nc.tensor.matmul(av[:, o:QT],
                                                 vaug[:, kc, h, :],
                                                 pt[:, j, o:QT],
                                                 start=(kc == 0),
                                                 stop=(kc == nkc - 1))
                        nc.vector.tensor_copy(
                            out=yTun[:, qj * HPC + h, :], in_=av[:])
                        if h == 1:
                            rt_half0 = recip_half(qj, 0)
                    cur = (qj, rt_half0 + recip_half(qj, 1))
                    if prev_qj is not None:
                        normmul(*prev_qj)
                    if qj >= 2:
                        proj_slice(qj - 2)
                    prev_qj = cur
                proj_slice(2)
                normmul(*prev_qj)
                proj_slice(3)
                for p in reversed(attn_scope):
                    p.__exit__(None, None, None)
    return nc


# ---------------------------------------------------------------- host

_NC_CACHE = []


def _get_nc():
    if not _NC_CACHE:
        _install_bir_fix()
        _NC_CACHE.append(build())
    return _NC_CACHE[0]


def make_in_maps(x, w_attn, w_proj):
    in_maps = []
    for c in range(N_CORES):
        b, h0 = c // 4, (c % 4) * HPC
        wq = w_attn[:, h0 * D:(h0 + HPC) * D]
        wk = w_attn[:, E + h0 * D:E + (h0 + HPC) * D]
        wv = w_attn[:, 2 * E + h0 * D:2 * E + (h0 + HPC) * D]
        in_maps.append({
            "xT": np.ascontiguousarray(x[b].T),
            "wqk": np.ascontiguousarray(np.concatenate([wq, wk], axis=1)),
            "wv": np.ascontiguousarray(wv),
            "wproj": np.ascontiguousarray(w_proj[h0 * D:(h0 + HPC) * D, :]),
        })
    return in_maps


def run(x, w_attn, w_proj, trace=False, tmpdir=None):
    from concourse.bass_utils import run_bass_kernel_spmd
    nc = _get_nc()
    res = run_bass_kernel_spmd(nc, make_in_maps(x, w_attn, w_proj),
                               list(range(N_CORES)), trace=trace, tmpdir=tmpdir)
    y = np.zeros((B, S, E), np.float32)
    for c in range(N_CORES):
        y[c // 4] += res.results[c]["y"]
    return y, res


def kernel(x, w_attn, w_proj):
    y, _ = run(np.asarray(x, np.float32), np.asarray(w_attn, np.float32),
               np.asarray(w_proj, np.float32))
    return y
